# revision 44
# baseline (speedup 1.0000x reference)
"""Trainium2 Bass kernel for nn_Attention_37598143710100.

Full attention layer: qkv proj -> rms norm q,k -> rope -> softmax attention
-> out proj.  B=4, L=4096, C=1024, H=16, D=64.

Sharding: 8 cores = (batch b in 0..3) x (query half qh in 0..1).  Each core
computes out[b, qh*2048:(qh+1)*2048, :] completely; host concatenates.
Inside a core the key/value positions are permuted to [own-half | other-half]
so the SPMD program is identical across cores (softmax is order-invariant).

v3: pair-major software pipeline.  x is resident in SBUF (no DMA in the proj
inner loops); PSUM is statically partitioned (scores 2 banks + ctx accum 3 +
proj 2 + small 1 = 8) so projection of pair N+1 interleaves into the
ACT(exp)-bound attention of pair N, keeping the PE dense and HAM-warm.
V is projected directly transposed (lhsT = x chunk), dropping the PE
transposes.  RMS rsqrt via DVE Newton; reciprocal_approx_fast for softmax
denominators; ctx bounced through DRAM for the output projection.
"""

import numpy as np
import ml_dtypes

B, L, C, H, D = 4, 4096, 1024, 16, 64
NCORES = 8
LQ = L // 2
EPS = 1e-6
NPAIR = H // 2  # 8
NQC = 4         # query chunks of 512 per core
RSQ_A, RSQ_B = 1.3750, 0.2700  # rsqrt Newton init y0 = A - B*x on [0.4, 3.5]

_compiled = None


def _build():
    import concourse.tile as tile
    from concourse import bacc, mybir

    bf16 = mybir.dt.bfloat16
    f32 = mybir.dt.float32
    AF = mybir.ActivationFunctionType

    nc = bacc.Bacc("TRN2", target_bir_lowering=False, debug=False,
                   enable_asserts=True, num_devices=NCORES)

    xT = nc.dram_tensor("xT", [C, L], bf16, kind="ExternalInput").ap()
    wT = nc.dram_tensor("wT", [C, 3 * C], bf16, kind="ExternalInput").ap()
    wpT = nc.dram_tensor("wpT", [C, C], bf16, kind="ExternalInput").ap()
    cgq = nc.dram_tensor("cgq", [128, LQ], bf16, kind="ExternalInput").ap()
    sgq = nc.dram_tensor("sgq", [128, LQ], bf16, kind="ExternalInput").ap()
    cgk = nc.dram_tensor("cgk", [128, L], bf16, kind="ExternalInput").ap()
    sgk = nc.dram_tensor("sgk", [128, L], bf16, kind="ExternalInput").ap()
    onesAB = nc.dram_tensor("onesAB", [128, 2], bf16, kind="ExternalInput").ap()
    bproj = nc.dram_tensor("bproj", [1, C], f32, kind="ExternalInput").ap()
    out_ap = nc.dram_tensor("out", [LQ, C], f32, kind="ExternalOutput").ap()

    # scratch: rms inv rows (q: 2jc+h, k: 8+2jc+h), softmax recips, ctx bounce
    inv_sc = nc.dram_tensor("inv_sc", [NPAIR, 24, 512], bf16).ap()
    rcp_sc = nc.dram_tensor("rcp_sc", [NPAIR, NQC, 2, 512], bf16).ap()
    ctx_sc = nc.dram_tensor("ctx_sc", [NPAIR, 128, LQ], bf16).ap()

    xTr = xT.rearrange("(eb p) j -> p eb j", p=128)
    wTr = wT.rearrange("(eb p) f -> p eb f", p=128)
    wpTr = wpT.rearrange("(cb p) o -> p cb o", p=128)

    with tile.TileContext(nc) as tc:
        with tc.tile_pool(name="persist", bufs=1) as persist:
            onesT = persist.tile([128, 2], bf16, tag="onesT")
            nc.sync.dma_start(onesT[:], onesAB[:])
            bp_b = persist.tile([128, C], f32, tag="bp")
            nc.sync.dma_start(bp_b[:], bproj[0:1, :].partition_broadcast(128))
            xsb = persist.tile([128, 8, L], bf16, tag="x")
            for jq in range(8):
                sl = slice(jq * 512, (jq + 1) * 512)
                for eb in range(8):
                    eng = nc.gpsimd if (eb % 2) else nc.sync
                    eng.dma_start(xsb[:, eb, sl], xTr[:, eb, sl])
            cgq_b = persist.tile([128, LQ], bf16, tag="cgq")
            sgq_b = persist.tile([128, LQ], bf16, tag="sgq")
            cgk_b = persist.tile([128, L], bf16, tag="cgk")
            sgk_b = persist.tile([128, L], bf16, tag="sgk")
            for ti, (t, src, n) in enumerate(((cgq_b, cgq, 4), (sgq_b, sgq, 4),
                                              (cgk_b, cgk, 8), (sgk_b, sgk, 8))):
                for jq in range(n):
                    sl = slice(jq * 512, (jq + 1) * 512)
                    eng = nc.gpsimd if ((ti + jq) % 2) else nc.sync
                    eng.dma_start(t[:, sl], src[:, sl])

            state = {}  # hp -> (qhat, khat, vsb)

            with tc.tile_pool(name="wslp", bufs=2) as wslp, \
                 tc.tile_pool(name="raws", bufs=1) as raws, \
                 tc.tile_pool(name="pairq", bufs=2) as pairq, \
                 tc.tile_pool(name="pairk", bufs=2) as pairk, \
                 tc.tile_pool(name="pairv", bufs=2) as pairv, \
                 tc.tile_pool(name="work", bufs=2) as work, \
                 tc.tile_pool(name="nwt", bufs=1) as nwt, \
                 tc.tile_pool(name="exps", bufs=2) as exps, \
                 tc.tile_pool(name="ctfp", bufs=1) as ctfp, \
                 tc.tile_pool(name="wpp", bufs=1) as wpp, \
                 tc.tile_pool(name="scp", bufs=2, space="PSUM") as scp, \
                 tc.tile_pool(name="ctxp", bufs=2, space="PSUM") as ctxp, \
                 tc.tile_pool(name="pp", bufs=2, space="PSUM") as pp:

                def proj_steps(hp):
                    """List of closures emitting pair hp's projection,
                    norm+rope, and transposed V; interleaved into the
                    previous pair's attention."""
                    S = []
                    st = {}

                    def s_setup():
                        w_sl = wslp.tile([128, 8, 3, 128], bf16, tag="wsl",
                                         name="w_sl")
                        for t in range(3):
                            nc.sync.dma_start(
                                w_sl[:, :, t, :],
                                wTr[:, :, t * C + hp * 128:
                                    t * C + (hp + 1) * 128])
                        st["w"] = w_sl
                        st["coll"] = raws.tile([24, 512], bf16, tag="coll",
                                               name="coll")
                        # q is stored zero-padded per head (qz0 rows 64:128
                        # and qz1 rows 0:64 stay zero) so the scores matmul
                        # contracts a full 128-row khat block without
                        # tile_position — avoids PE weight-geometry switches
                        qz0 = pairq.tile([128, LQ], bf16, tag="qz0",
                                         name=f"qz0_{hp}")
                        qz1 = pairq.tile([128, LQ], bf16, tag="qz1",
                                         name=f"qz1_{hp}")
                        nc.gpsimd.memset(qz0[64:128, :], 0.0)
                        nc.gpsimd.memset(qz1[0:64, :], 0.0)
                        kh = pairk.tile([128, L], bf16, tag="khat",
                                        name=f"khat{hp}")
                        vs = pairv.tile([128, 32, 2, 65], bf16, tag="vsb",
                                        name=f"vsb{hp}")
                        nc.vector.memset(vs[:, :, :, 64:65], 1.0)
                        state[hp] = ((qz0, qz1), kh, vs)
                    S.append(s_setup)

                    def mk_kq_a(tsr, jc):
                        def f():
                            sl = slice(jc * 512, (jc + 1) * 512)
                            ps = pp.tile([128, 512], f32, tag="pp", name="ps")
                            st["ps"] = ps
                            for eb in range(4):
                                nc.tensor.matmul(ps[:], st["w"][:, eb, tsr, :],
                                                 xsb[:, eb, sl],
                                                 start=(eb == 0), stop=False)
                        return f

                    def mk_kq_b(tsr, jc):
                        def f():
                            sl = slice(jc * 512, (jc + 1) * 512)
                            ps = st["ps"]
                            for eb in range(4, 8):
                                nc.tensor.matmul(ps[:], st["w"][:, eb, tsr, :],
                                                 xsb[:, eb, sl],
                                                 start=False, stop=(eb == 7))
                            # raw q/k lands in qz0/qz1/khat; rope is in-place
                            sq = work.tile([128, 512], bf16, tag="sq",
                                           name="sq", bufs=1)
                            if tsr == 0:
                                qz0, qz1 = state[hp][0]
                                nc.vector.tensor_copy(qz0[0:64, sl],
                                                      ps[0:64, :])
                                nc.vector.tensor_copy(qz1[64:128, sl],
                                                      ps[64:128, :])
                                nc.vector.tensor_mul(sq[0:64, :],
                                                     qz0[0:64, sl],
                                                     qz0[0:64, sl])
                                nc.vector.tensor_mul(sq[64:128, :],
                                                     qz1[64:128, sl],
                                                     qz1[64:128, sl])
                            else:
                                raw = state[hp][1]
                                nc.vector.tensor_copy(raw[:, sl], ps[:])
                                nc.vector.tensor_mul(sq[:], raw[:, sl],
                                                     raw[:, sl])
                            pss = pp.tile([2, 512], f32, tag="pp",
                                          name="pss")
                            nc.tensor.matmul(pss[:], onesT[:], sq[:],
                                             start=True, stop=True)
                            cp2 = work.tile([2, 512], bf16, tag="cp2",
                                            name="cp2", bufs=1)
                            nc.vector.tensor_copy(cp2[:], pss[:])
                            r0 = 2 * jc if tsr == 0 else 8 + 2 * jc
                            nc.sync.dma_start(st["coll"][r0:r0 + 2, :], cp2[:])
                        return f

                    for jc in range(8):
                        S.append(mk_kq_a(1, jc))
                        S.append(mk_kq_b(1, jc))
                    for jc in range(4):
                        S.append(mk_kq_a(0, jc))
                        S.append(mk_kq_b(0, jc))

                    def s_newton():
                        # rsqrt(ms+eps) via Newton on DVE: x = coll/64 + eps
                        xms = nwt.tile([24, 512], f32, tag="xms", name="xms")
                        nc.vector.tensor_scalar(
                            xms[:], st["coll"][:], 1.0 / 64.0, EPS,
                            op0=mybir.AluOpType.mult, op1=mybir.AluOpType.add)
                        y = nwt.tile([24, 512], f32, tag="y", name="y")
                        nc.vector.tensor_scalar(
                            y[:], xms[:], -RSQ_B, RSQ_A,
                            op0=mybir.AluOpType.mult, op1=mybir.AluOpType.add)
                        t1 = nwt.tile([24, 512], f32, tag="t1", name="t1")
                        inv24 = nwt.tile([24, 512], bf16, tag="inv24",
                                         name="inv24")
                        for it in range(3):
                            nc.vector.tensor_mul(t1[:], xms[:], y[:])
                            nc.vector.tensor_mul(t1[:], t1[:], y[:])
                            nc.vector.tensor_scalar(
                                t1[:], t1[:], -0.5, 1.5,
                                op0=mybir.AluOpType.mult,
                                op1=mybir.AluOpType.add)
                            if it < 2:
                                nc.vector.tensor_mul(y[:], y[:], t1[:])
                            else:
                                nc.vector.tensor_mul(inv24[:], y[:], t1[:])
                        nc.sync.dma_start(inv_sc[hp, :, :], inv24[:])
                    S.append(s_newton)

                    def mk_rope(tsr, jc):
                        # hat = raw*inv*cg + rot32(raw)*inv*sg  (per chunk)
                        def f():
                            sl = slice(jc * 512, (jc + 1) * 512)
                            if tsr == 0:
                                qz0, qz1 = state[hp][0]
                                half = {0: qz0, 1: qz1}
                            else:
                                kh_t = state[hp][1]
                                half = {0: kh_t, 1: kh_t}
                            cg_b = cgq_b if tsr == 0 else cgk_b
                            sg_b = sgq_b if tsr == 0 else sgk_b
                            r0 = 2 * jc if tsr == 0 else 8 + 2 * jc
                            shf = work.tile([128, 512], bf16, tag="shf",
                                            name="shf")
                            for a, b in ((0, 32), (32, 0), (64, 96), (96, 64)):
                                src = half[0] if b < 64 else half[1]
                                nc.gpsimd.dma_start(shf[a:a + 32, :],
                                                    src[b:b + 32, sl])
                            ib = work.tile([128, 512], bf16, tag="ib",
                                           name="ib", bufs=1)
                            nc.sync.dma_start(
                                ib[0:64, :],
                                inv_sc[hp, r0:r0 + 1, :]
                                .partition_broadcast(64))
                            nc.sync.dma_start(
                                ib[64:128, :],
                                inv_sc[hp, r0 + 1:r0 + 2, :]
                                .partition_broadcast(64))
                            icg = work.tile([128, 512], bf16, tag="icg",
                                            name="icg")
                            nc.vector.tensor_mul(icg[:], ib[:], cg_b[:, sl])
                            isg = work.tile([128, 512], bf16, tag="isg",
                                            name="isg")
                            nc.vector.tensor_mul(isg[:], ib[:], sg_b[:, sl])
                            u = work.tile([128, 512], bf16, tag="u", name="u")
                            v2 = work.tile([128, 512], bf16, tag="v2",
                                           name="v2")
                            nc.vector.tensor_mul(v2[:], shf[:], isg[:])
                            if tsr == 0:
                                nc.vector.tensor_mul(u[0:64, :],
                                                     half[0][0:64, sl],
                                                     icg[0:64, :])
                                nc.vector.tensor_mul(u[64:128, :],
                                                     half[1][64:128, sl],
                                                     icg[64:128, :])
                                nc.vector.tensor_add(half[0][0:64, sl],
                                                     u[0:64, :], v2[0:64, :])
                                nc.vector.tensor_add(half[1][64:128, sl],
                                                     u[64:128, :],
                                                     v2[64:128, :])
                            else:
                                nc.vector.tensor_mul(u[:], half[0][:, sl],
                                                     icg[:])
                                nc.vector.tensor_add(half[0][:, sl], u[:],
                                                     v2[:])
                        return f

                    rope_steps = [mk_rope(1, jc) for jc in range(8)]
                    ropeq_steps = [mk_rope(0, jc) for jc in range(4)]

                    def mk_v(vj):
                        # V projected directly transposed: out[pos, dims]
                        def f():
                            for c in range(2):
                                jg = vj * 2 + c
                                vt = pp.tile([128, 128], f32, tag="pp",
                                             name="vt")
                                for eb in range(8):
                                    nc.tensor.matmul(
                                        vt[:],
                                        xsb[:, eb, jg * 128:(jg + 1) * 128],
                                        st["w"][:, eb, 2, :],
                                        start=(eb == 0), stop=(eb == 7))
                                nc.vector.tensor_copy(
                                    state[hp][2][:, jg, :, 0:64],
                                    vt[:].rearrange("p (h d) -> p h d", h=2))
                        return f

                    # attention-critical chain first: khat/qhat/vsb chunk 0
                    v_steps = [mk_v(vj) for vj in range(16)]
                    S.append(rope_steps[0])
                    S.append(ropeq_steps[0])
                    S.append(v_steps[0])
                    for jc in range(1, 8):
                        S.append(rope_steps[jc])
                        S.append(v_steps[jc])
                    S += ropeq_steps[1:]
                    S += v_steps[8:]
                    return S

                def ctx_epilogue(hp, ip, h, ctxt):
                    ctf = ctfp.tile([65, 512], bf16, tag="ctf", name="ctf")
                    nc.vector.tensor_copy(ctf[:], ctxt[:])
                    rcs = work.tile([1, 512], f32, tag="rcs", name="rcs",
                                    bufs=1)
                    nc.vector.tensor_copy(rcs[:], ctf[64:65, :])
                    rcp = work.tile([1, 512], f32, tag="rcp", name="rcp",
                                    bufs=1)
                    nc.vector.reciprocal_approx_fast(out=rcp[:], in_=rcs[:])
                    rcpb = work.tile([1, 512], bf16, tag="rcpb", name="rcpb",
                                     bufs=1)
                    nc.vector.tensor_copy(rcpb[:], rcp[:])
                    nc.sync.dma_start(rcp_sc[hp, ip, h:h + 1, :], rcpb[0:1, :])
                    rb = work.tile([64, 512], bf16, tag="rb", name="rb",
                                   bufs=2)
                    nc.sync.dma_start(
                        rb[:],
                        rcp_sc[hp, ip, h:h + 1, :].partition_broadcast(64))
                    cto = work.tile([64, 512], bf16, tag="cto", name="cto",
                                    bufs=2)
                    nc.vector.tensor_mul(cto[:], ctf[0:64, :], rb[:])
                    nc.sync.dma_start(
                        ctx_sc[hp, h * 64:(h + 1) * 64,
                               ip * 512:(ip + 1) * 512], cto[:])

                wpc = {}

                def wpc_steps():
                    # resident W_proj slices, loaded during pair-7 ip0
                    def mk(cb):
                        def f():
                            if "t" not in wpc:
                                wpc["t"] = wpp.tile([128, 8, C], bf16,
                                                    tag="wpc", name="wpc")
                            wpc[cb] = wpc["t"][:, cb, :]
                            eng = nc.gpsimd if (cb % 2) else nc.sync
                            eng.dma_start(wpc[cb], wpTr[:, cb, :])
                        return f
                    return [mk(cb) for cb in range(8)]

                def outproj_steps(ibb, alt_pool=False):
                    S = []
                    st2 = {}
                    for i in range(4):
                        ib = ibb * 4 + i
                        # tail group: odd ibs borrow the idle score-pool banks
                        # so two ibs accumulate concurrently
                        opool, otag = ((scp, "sc") if (alt_pool and i % 2)
                                       else (pp, "pp"))

                        def mk_a(ib=ib, opool=opool, otag=otag):
                            def f():
                                a0 = opool.tile([128, 512], f32, tag=otag,
                                                name="oacc0")
                                a1 = opool.tile([128, 512], f32, tag=otag,
                                                name="oacc1")
                                st2[ib] = (a0, a1)
                                for cb in range(4):
                                    ct = work.tile([128, 128], bf16, tag="ct",
                                                   name="ct")
                                    nc.sync.dma_start(
                                        ct[:],
                                        ctx_sc[cb, :,
                                               ib * 128:(ib + 1) * 128])
                                    nc.tensor.matmul(
                                        a0[:], ct[:], wpc[cb][:, 0:512],
                                        start=(cb == 0), stop=False)
                                    nc.tensor.matmul(
                                        a1[:], ct[:], wpc[cb][:, 512:1024],
                                        start=(cb == 0), stop=False)
                            return f

                        def mk_b(ib=ib):
                            def f():
                                a0, a1 = st2[ib]
                                for cb in range(4, 8):
                                    ct = work.tile([128, 128], bf16, tag="ct",
                                                   name="ct")
                                    nc.sync.dma_start(
                                        ct[:],
                                        ctx_sc[cb, :,
                                               ib * 128:(ib + 1) * 128])
                                    nc.tensor.matmul(
                                        a0[:], ct[:], wpc[cb][:, 0:512],
                                        start=False, stop=(cb == 7))
                                    nc.tensor.matmul(
                                        a1[:], ct[:], wpc[cb][:, 512:1024],
                                        start=False, stop=(cb == 7))
                                for half, acc in ((0, a0), (1, a1)):
                                    ot = work.tile([128, 512], f32, tag="ot",
                                                   name="ot")
                                    nc.vector.tensor_add(
                                        ot[:], acc[:],
                                        bp_b[:, half * 512:(half + 1) * 512])
                                    nc.sync.dma_start(
                                        out_ap[ib * 128:(ib + 1) * 128,
                                               half * 512:(half + 1) * 512],
                                        ot[:])
                            return f

                        S.append(mk_a())
                        S.append(mk_b())
                    return S

                def attention(hp, steps_by_ip):
                    qzs, kh, vs = state[hp]
                    ctx = {}
                    for ip in range(NQC):
                        qsl = slice(ip * 512, (ip + 1) * 512)
                        steps = steps_by_ip[ip]
                        k = 0
                        for jp in range(16):
                            sc = {}
                            ee = {}
                            for h in range(2):
                                sc[h] = scp.tile([128, 2, 512], f32,
                                                 tag="sc", name="sc")
                                for jj in range(2):
                                    j = jp * 2 + jj
                                    nc.tensor.matmul(
                                        sc[h][:, jj, :],
                                        kh[:, j * 128:(j + 1) * 128],
                                        qzs[h][:, qsl],
                                        start=True, stop=True)
                            for h in range(2):
                                ee[h] = exps.tile([128, 2, 512], bf16,
                                                  tag="e", name="e")
                                nc.scalar.activation(ee[h][:], sc[h][:],
                                                     AF.Exp, scale=0.125)
                            for h in range(2):
                                if jp == 0:
                                    ctx[h] = ctxp.tile([65, 512], f32,
                                                       tag="ctx",
                                                       name=f"ctx{h}")
                                for jj in range(2):
                                    j = jp * 2 + jj
                                    nc.tensor.matmul(
                                        ctx[h][:], vs[:, j, h, :],
                                        ee[h][:, jj, :], start=(j == 0),
                                        stop=(j == 31))
                            if jp == 15:
                                for h in range(2):
                                    ctx_epilogue(hp, ip, h, ctx[h])
                            while k < len(steps) * (jp + 1) // 16:
                                steps[k]()
                                k += 1

                def split4(S):
                    # spread one step list across the 4 ip sub-loops
                    return [S[len(S) * i // 4: len(S) * (i + 1) // 4]
                            for i in range(4)]

                for f in proj_steps(0):
                    f()
                for hp in range(NPAIR):
                    if hp + 1 < NPAIR:
                        steps_by_ip = split4(proj_steps(hp + 1))
                    else:
                        steps_by_ip = [wpc_steps(), outproj_steps(0),
                                       outproj_steps(1), outproj_steps(2)]
                    attention(hp, steps_by_ip)
                for f in outproj_steps(3, alt_pool=True):
                    f()

    nc.compile()
    return nc


def _host_prep(x, W_qkv, q_scale, k_scale, W_proj, b_proj, cos, sin):
    nbf = ml_dtypes.bfloat16
    cosn = np.asarray(cos, np.float32)
    sinn = np.asarray(sin, np.float32)
    qs = np.asarray(q_scale, np.float32)
    ks = np.asarray(k_scale, np.float32)

    def tables(g):
        sign = np.concatenate([-np.ones(D // 2), np.ones(D // 2)]).astype(np.float32)
        gpart = np.concatenate([g[D // 2:], g[:D // 2]])
        cg = cosn * g[None, :]
        sg = sinn * (sign * gpart)[None, :]
        return cg.T.copy(), sg.T.copy()  # [D, L]

    cgq_f, sgq_f = tables(qs)
    cgk_f, sgk_f = tables(ks)

    def b128(t):  # [64, n] -> [128, n] (two heads stacked)
        return np.ascontiguousarray(np.concatenate([t, t], axis=0)).astype(nbf)

    wT = np.asarray(W_qkv, np.float32).T.astype(nbf)
    wpT = np.asarray(W_proj, np.float32).T.astype(nbf)
    bp = np.asarray(b_proj, np.float32).reshape(1, C)
    onesAB = np.zeros((128, 2), nbf)
    onesAB[0:64, 0] = 1.0
    onesAB[64:128, 1] = 1.0

    xn = np.asarray(x, np.float32)
    in_maps = []
    for core in range(NCORES):
        b, qh = core // 2, core % 2
        own = slice(qh * LQ, (qh + 1) * LQ)
        perm = np.r_[np.arange(qh * LQ, (qh + 1) * LQ),
                     np.arange((1 - qh) * LQ, (2 - qh) * LQ)]
        xTc = xn[b].T[:, perm].astype(nbf)
        in_maps.append({
            "xT": np.ascontiguousarray(xTc),
            "wT": wT, "wpT": wpT,
            "cgq": b128(cgq_f[:, own]),
            "sgq": b128(sgq_f[:, own]),
            "cgk": b128(cgk_f[:, perm]),
            "sgk": b128(sgk_f[:, perm]),
            "onesAB": onesAB, "bproj": bp,
        })
    return in_maps


def kernel(x, W_qkv, q_scale, k_scale, W_proj, b_proj, cos, sin, _trace=False):
    global _compiled
    from concourse.bass_utils import run_bass_kernel_spmd
    if _compiled is None:
        _compiled = _build()
    in_maps = _host_prep(x, W_qkv, q_scale, k_scale, W_proj, b_proj, cos, sin)
    res = run_bass_kernel_spmd(_compiled, in_maps, core_ids=list(range(NCORES)),
                               trace=_trace)
    out = np.empty((B, L, C), np.float32)
    for core in range(NCORES):
        b, qh = core // 2, core % 2
        out[b, qh * LQ:(qh + 1) * LQ, :] = res.results[core]["out"]
    kernel._last = res
    return out


# revision 45
# speedup vs baseline: 1.0646x; 1.0646x over previous
"""Trainium2 Bass kernel for nn_Attention_37598143710100.

Full attention layer: qkv proj -> rms norm q,k -> rope -> softmax attention
-> out proj.  B=4, L=4096, C=1024, H=16, D=64.

Sharding: 8 cores = (batch b in 0..3) x (query half qh in 0..1).  Each core
computes out[b, qh*2048:(qh+1)*2048, :] completely; host concatenates.
Inside a core the key/value positions are permuted to [own-half | other-half]
so the SPMD program is identical across cores (softmax is order-invariant).

v3: pair-major software pipeline.  x is resident in SBUF (no DMA in the proj
inner loops); PSUM is statically partitioned (scores 2 banks + ctx accum 3 +
proj 2 + small 1 = 8) so projection of pair N+1 interleaves into the
ACT(exp)-bound attention of pair N, keeping the PE dense and HAM-warm.
V is projected directly transposed (lhsT = x chunk), dropping the PE
transposes.  RMS rsqrt via DVE Newton; reciprocal_approx_fast for softmax
denominators; ctx bounced through DRAM for the output projection.
"""

import numpy as np
import ml_dtypes

B, L, C, H, D = 4, 4096, 1024, 16, 64
NCORES = 8
LQ = L // 2
EPS = 1e-6
NPAIR = H // 2  # 8
NQC = 4         # query chunks of 512 per core
RSQ_A, RSQ_B = 1.3750, 0.2700  # rsqrt Newton init y0 = A - B*x on [0.4, 3.5]

_compiled = None


def _build():
    import concourse.tile as tile
    from concourse import bacc, mybir

    bf16 = mybir.dt.bfloat16
    f32 = mybir.dt.float32
    AF = mybir.ActivationFunctionType

    nc = bacc.Bacc("TRN2", target_bir_lowering=False, debug=False,
                   enable_asserts=True, num_devices=NCORES)

    xT = nc.dram_tensor("xT", [C, L], bf16, kind="ExternalInput").ap()
    wT = nc.dram_tensor("wT", [C, 3 * C], bf16, kind="ExternalInput").ap()
    wpT = nc.dram_tensor("wpT", [C, C], bf16, kind="ExternalInput").ap()
    cgq = nc.dram_tensor("cgq", [128, LQ], bf16, kind="ExternalInput").ap()
    sgq = nc.dram_tensor("sgq", [128, LQ], bf16, kind="ExternalInput").ap()
    cgk = nc.dram_tensor("cgk", [128, L], bf16, kind="ExternalInput").ap()
    sgk = nc.dram_tensor("sgk", [128, L], bf16, kind="ExternalInput").ap()
    onesAB = nc.dram_tensor("onesAB", [128, 2], bf16, kind="ExternalInput").ap()
    bproj = nc.dram_tensor("bproj", [1, C], f32, kind="ExternalInput").ap()
    out_ap = nc.dram_tensor("out", [LQ, C], f32, kind="ExternalOutput").ap()

    # scratch: rms inv rows (q: 2jc+h, k: 8+2jc+h), softmax recips, ctx bounce
    inv_sc = nc.dram_tensor("inv_sc", [NPAIR, 24, 512], bf16).ap()
    rcp_sc = nc.dram_tensor("rcp_sc", [NPAIR, NQC, 2, 512], bf16).ap()
    ctx_sc = nc.dram_tensor("ctx_sc", [NPAIR, 128, LQ], bf16).ap()

    xTr = xT.rearrange("(eb p) j -> p eb j", p=128)
    wTr = wT.rearrange("(eb p) f -> p eb f", p=128)
    wpTr = wpT.rearrange("(cb p) o -> p cb o", p=128)

    with tile.TileContext(nc) as tc:
        with tc.tile_pool(name="persist", bufs=1) as persist:
            onesT = persist.tile([128, 2], bf16, tag="onesT")
            nc.sync.dma_start(onesT[:], onesAB[:])
            bp_b = persist.tile([128, C], f32, tag="bp")
            nc.sync.dma_start(bp_b[:], bproj[0:1, :].partition_broadcast(128))
            xsb = persist.tile([128, 8, L], bf16, tag="x")
            for jq in range(8):
                sl = slice(jq * 512, (jq + 1) * 512)
                for eb in range(8):
                    eng = nc.gpsimd if (eb % 2) else nc.sync
                    eng.dma_start(xsb[:, eb, sl], xTr[:, eb, sl])
            cgq_b = persist.tile([128, LQ], bf16, tag="cgq")
            sgq_b = persist.tile([128, LQ], bf16, tag="sgq")
            cgk_b = persist.tile([128, L], bf16, tag="cgk")
            sgk_b = persist.tile([128, L], bf16, tag="sgk")
            for ti, (t, src, n) in enumerate(((cgq_b, cgq, 4), (sgq_b, sgq, 4),
                                              (cgk_b, cgk, 8), (sgk_b, sgk, 8))):
                for jq in range(n):
                    sl = slice(jq * 512, (jq + 1) * 512)
                    eng = nc.gpsimd if ((ti + jq) % 2) else nc.sync
                    eng.dma_start(t[:, sl], src[:, sl])

            state = {}  # hp -> (qhat, khat, vsb)

            with tc.tile_pool(name="wslp", bufs=2) as wslp, \
                 tc.tile_pool(name="raws", bufs=1) as raws, \
                 tc.tile_pool(name="pairq", bufs=2) as pairq, \
                 tc.tile_pool(name="pairk", bufs=2) as pairk, \
                 tc.tile_pool(name="pairv", bufs=2) as pairv, \
                 tc.tile_pool(name="work", bufs=2) as work, \
                 tc.tile_pool(name="nwt", bufs=1) as nwt, \
                 tc.tile_pool(name="exps", bufs=3) as exps, \
                 tc.tile_pool(name="ctfp", bufs=2) as ctfp, \
                 tc.tile_pool(name="wpp", bufs=1) as wpp, \
                 tc.tile_pool(name="scp", bufs=2, space="PSUM") as scp, \
                 tc.tile_pool(name="ctxp", bufs=2, space="PSUM") as ctxp, \
                 tc.tile_pool(name="pp", bufs=2, space="PSUM") as pp:

                def proj_steps(hp):
                    """List of closures emitting pair hp's projection,
                    norm+rope, and transposed V; interleaved into the
                    previous pair's attention."""
                    S = []
                    st = {}

                    def s_setup():
                        w_sl = wslp.tile([128, 8, 3, 128], bf16, tag="wsl",
                                         name="w_sl")
                        for t in range(3):
                            nc.sync.dma_start(
                                w_sl[:, :, t, :],
                                wTr[:, :, t * C + hp * 128:
                                    t * C + (hp + 1) * 128])
                        st["w"] = w_sl
                        st["coll"] = raws.tile([24, 512], bf16, tag="coll",
                                               name="coll")
                        # q is stored zero-padded per head (qz0 rows 64:128
                        # and qz1 rows 0:64 stay zero) so the scores matmul
                        # contracts a full 128-row khat block without
                        # tile_position — avoids PE weight-geometry switches
                        qz0 = pairq.tile([128, LQ], bf16, tag="qz0",
                                         name=f"qz0_{hp}")
                        qz1 = pairq.tile([128, LQ], bf16, tag="qz1",
                                         name=f"qz1_{hp}")
                        nc.gpsimd.memset(qz0[64:128, :], 0.0)
                        nc.gpsimd.memset(qz1[0:64, :], 0.0)
                        kh = pairk.tile([128, L], bf16, tag="khat",
                                        name=f"khat{hp}")
                        vs = pairv.tile([128, 32, 2, 65], bf16, tag="vsb",
                                        name=f"vsb{hp}")
                        nc.vector.memset(vs[:, :, :, 64:65], 1.0)
                        state[hp] = ((qz0, qz1), kh, vs)
                    S.append(s_setup)

                    def mk_kq_a(tsr, jc):
                        def f():
                            sl = slice(jc * 512, (jc + 1) * 512)
                            ps = pp.tile([128, 512], f32, tag="pp", name="ps")
                            st["ps"] = ps
                            for eb in range(4):
                                nc.tensor.matmul(ps[:], st["w"][:, eb, tsr, :],
                                                 xsb[:, eb, sl],
                                                 start=(eb == 0), stop=False)
                        return f

                    def mk_kq_b(tsr, jc):
                        def f():
                            sl = slice(jc * 512, (jc + 1) * 512)
                            ps = st["ps"]
                            for eb in range(4, 8):
                                nc.tensor.matmul(ps[:], st["w"][:, eb, tsr, :],
                                                 xsb[:, eb, sl],
                                                 start=False, stop=(eb == 7))
                            # raw q/k lands in qz0/qz1/khat; rope is in-place
                            sq = work.tile([128, 512], bf16, tag="sq",
                                           name="sq", bufs=1)
                            if tsr == 0:
                                qz0, qz1 = state[hp][0]
                                nc.vector.tensor_copy(qz0[0:64, sl],
                                                      ps[0:64, :])
                                nc.vector.tensor_copy(qz1[64:128, sl],
                                                      ps[64:128, :])
                                nc.vector.tensor_mul(sq[0:64, :],
                                                     qz0[0:64, sl],
                                                     qz0[0:64, sl])
                                nc.vector.tensor_mul(sq[64:128, :],
                                                     qz1[64:128, sl],
                                                     qz1[64:128, sl])
                            else:
                                raw = state[hp][1]
                                nc.vector.tensor_copy(raw[:, sl], ps[:])
                                nc.vector.tensor_mul(sq[:], raw[:, sl],
                                                     raw[:, sl])
                            pss = pp.tile([2, 512], f32, tag="pp",
                                          name="pss")
                            nc.tensor.matmul(pss[:], onesT[:], sq[:],
                                             start=True, stop=True)
                            cp2 = work.tile([2, 512], bf16, tag="cp2",
                                            name="cp2", bufs=1)
                            nc.vector.tensor_copy(cp2[:], pss[:])
                            r0 = 2 * jc if tsr == 0 else 8 + 2 * jc
                            nc.sync.dma_start(st["coll"][r0:r0 + 2, :], cp2[:])
                        return f

                    for jc in range(8):
                        S.append(mk_kq_a(1, jc))
                        S.append(mk_kq_b(1, jc))
                    for jc in range(4):
                        S.append(mk_kq_a(0, jc))
                        S.append(mk_kq_b(0, jc))

                    def s_newton():
                        # rsqrt(ms+eps) via Newton on DVE: x = coll/64 + eps
                        xms = nwt.tile([24, 512], f32, tag="xms", name="xms")
                        nc.vector.tensor_scalar(
                            xms[:], st["coll"][:], 1.0 / 64.0, EPS,
                            op0=mybir.AluOpType.mult, op1=mybir.AluOpType.add)
                        y = nwt.tile([24, 512], f32, tag="y", name="y")
                        nc.vector.tensor_scalar(
                            y[:], xms[:], -RSQ_B, RSQ_A,
                            op0=mybir.AluOpType.mult, op1=mybir.AluOpType.add)
                        t1 = nwt.tile([24, 512], f32, tag="t1", name="t1")
                        inv24 = nwt.tile([24, 512], bf16, tag="inv24",
                                         name="inv24")
                        for it in range(3):
                            nc.vector.tensor_mul(t1[:], xms[:], y[:])
                            nc.vector.tensor_mul(t1[:], t1[:], y[:])
                            nc.vector.tensor_scalar(
                                t1[:], t1[:], -0.5, 1.5,
                                op0=mybir.AluOpType.mult,
                                op1=mybir.AluOpType.add)
                            if it < 2:
                                nc.vector.tensor_mul(y[:], y[:], t1[:])
                            else:
                                nc.vector.tensor_mul(inv24[:], y[:], t1[:])
                        nc.sync.dma_start(inv_sc[hp, :, :], inv24[:])
                    S.append(s_newton)

                    def mk_rope(tsr, jc):
                        # hat = raw*inv*cg + rot32(raw)*inv*sg  (per chunk)
                        def f():
                            sl = slice(jc * 512, (jc + 1) * 512)
                            if tsr == 0:
                                qz0, qz1 = state[hp][0]
                                half = {0: qz0, 1: qz1}
                            else:
                                kh_t = state[hp][1]
                                half = {0: kh_t, 1: kh_t}
                            cg_b = cgq_b if tsr == 0 else cgk_b
                            sg_b = sgq_b if tsr == 0 else sgk_b
                            r0 = 2 * jc if tsr == 0 else 8 + 2 * jc
                            shf = work.tile([128, 512], bf16, tag="shf",
                                            name="shf", bufs=1)
                            for a, b in ((0, 32), (32, 0), (64, 96), (96, 64)):
                                src = half[0] if b < 64 else half[1]
                                nc.gpsimd.dma_start(shf[a:a + 32, :],
                                                    src[b:b + 32, sl])
                            ib = work.tile([128, 512], bf16, tag="ib",
                                           name="ib")
                            nc.sync.dma_start(
                                ib[0:64, :],
                                inv_sc[hp, r0:r0 + 1, :]
                                .partition_broadcast(64))
                            nc.sync.dma_start(
                                ib[64:128, :],
                                inv_sc[hp, r0 + 1:r0 + 2, :]
                                .partition_broadcast(64))
                            icg = work.tile([128, 512], bf16, tag="icg",
                                            name="icg", bufs=1)
                            nc.vector.tensor_mul(icg[:], ib[:], cg_b[:, sl])
                            isg = work.tile([128, 512], bf16, tag="isg",
                                            name="isg", bufs=1)
                            nc.vector.tensor_mul(isg[:], ib[:], sg_b[:, sl])
                            u = work.tile([128, 512], bf16, tag="u", name="u",
                                           bufs=1)
                            v2 = work.tile([128, 512], bf16, tag="v2",
                                           name="v2", bufs=1)
                            nc.vector.tensor_mul(v2[:], shf[:], isg[:])
                            if tsr == 0:
                                nc.vector.tensor_mul(u[0:64, :],
                                                     half[0][0:64, sl],
                                                     icg[0:64, :])
                                nc.vector.tensor_mul(u[64:128, :],
                                                     half[1][64:128, sl],
                                                     icg[64:128, :])
                                nc.vector.tensor_add(half[0][0:64, sl],
                                                     u[0:64, :], v2[0:64, :])
                                nc.vector.tensor_add(half[1][64:128, sl],
                                                     u[64:128, :],
                                                     v2[64:128, :])
                            else:
                                nc.vector.tensor_mul(u[:], half[0][:, sl],
                                                     icg[:])
                                nc.vector.tensor_add(half[0][:, sl], u[:],
                                                     v2[:])
                        return f

                    rope_steps = [mk_rope(1, jc) for jc in range(8)]
                    ropeq_steps = [mk_rope(0, jc) for jc in range(4)]

                    def mk_v(vj):
                        # V projected directly transposed: out[pos, dims]
                        def f():
                            for c in range(2):
                                jg = vj * 2 + c
                                vt = pp.tile([128, 128], f32, tag="pp",
                                             name="vt")
                                for eb in range(8):
                                    nc.tensor.matmul(
                                        vt[:],
                                        xsb[:, eb, jg * 128:(jg + 1) * 128],
                                        st["w"][:, eb, 2, :],
                                        start=(eb == 0), stop=(eb == 7))
                                nc.vector.tensor_copy(
                                    state[hp][2][:, jg, :, 0:64],
                                    vt[:].rearrange("p (h d) -> p h d", h=2))
                        return f

                    # attention-critical chain first: khat/qhat/vsb chunk 0
                    v_steps = [mk_v(vj) for vj in range(16)]
                    S.append(rope_steps[0])
                    S.append(ropeq_steps[0])
                    S.append(v_steps[0])
                    for jc in range(1, 8):
                        S.append(rope_steps[jc])
                        S.append(v_steps[jc])
                    S += ropeq_steps[1:]
                    S += v_steps[8:]
                    return S

                def ctx_epilogue(hp, ip, h, ctxt):
                    ctf = ctfp.tile([65, 512], bf16, tag="ctf", name="ctf")
                    nc.vector.tensor_copy(ctf[:], ctxt[:])
                    rcs = work.tile([1, 512], f32, tag="rcs", name="rcs",
                                    bufs=1)
                    nc.vector.tensor_copy(rcs[:], ctf[64:65, :])
                    rcp = work.tile([1, 512], f32, tag="rcp", name="rcp",
                                    bufs=1)
                    nc.vector.reciprocal_approx_fast(out=rcp[:], in_=rcs[:])
                    rcpb = work.tile([1, 512], bf16, tag="rcpb", name="rcpb",
                                     bufs=1)
                    nc.vector.tensor_copy(rcpb[:], rcp[:])
                    nc.sync.dma_start(rcp_sc[hp, ip, h:h + 1, :], rcpb[0:1, :])
                    rb = work.tile([64, 512], bf16, tag="rb", name="rb",
                                   bufs=2)
                    nc.sync.dma_start(
                        rb[:],
                        rcp_sc[hp, ip, h:h + 1, :].partition_broadcast(64))
                    cto = work.tile([64, 512], bf16, tag="cto", name="cto",
                                    bufs=2)
                    nc.vector.tensor_mul(cto[:], ctf[0:64, :], rb[:])
                    nc.sync.dma_start(
                        ctx_sc[hp, h * 64:(h + 1) * 64,
                               ip * 512:(ip + 1) * 512], cto[:])

                wpc = {}

                def wpc_steps():
                    # resident W_proj slices, loaded during pair-7 ip0
                    def mk(cb):
                        def f():
                            if "t" not in wpc:
                                wpc["t"] = wpp.tile([128, 8, C], bf16,
                                                    tag="wpc", name="wpc")
                            wpc[cb] = wpc["t"][:, cb, :]
                            eng = nc.gpsimd if (cb % 2) else nc.sync
                            eng.dma_start(wpc[cb], wpTr[:, cb, :])
                        return f
                    return [mk(cb) for cb in range(8)]

                def outproj_steps(ibb, alt_pool=False):
                    S = []
                    st2 = {}
                    for i in range(4):
                        ib = ibb * 4 + i
                        # tail group: odd ibs borrow the idle score-pool banks
                        # so two ibs accumulate concurrently
                        opool, otag = ((scp, "sc") if (alt_pool and i % 2)
                                       else (pp, "pp"))

                        def mk_a(ib=ib, opool=opool, otag=otag):
                            def f():
                                a0 = opool.tile([128, 512], f32, tag=otag,
                                                name="oacc0")
                                a1 = opool.tile([128, 512], f32, tag=otag,
                                                name="oacc1")
                                st2[ib] = (a0, a1)
                                for cb in range(4):
                                    ct = work.tile([128, 128], bf16, tag="ct",
                                                   name="ct")
                                    nc.sync.dma_start(
                                        ct[:],
                                        ctx_sc[cb, :,
                                               ib * 128:(ib + 1) * 128])
                                    nc.tensor.matmul(
                                        a0[:], ct[:], wpc[cb][:, 0:512],
                                        start=(cb == 0), stop=False)
                                    nc.tensor.matmul(
                                        a1[:], ct[:], wpc[cb][:, 512:1024],
                                        start=(cb == 0), stop=False)
                            return f

                        def mk_b(ib=ib):
                            def f():
                                a0, a1 = st2[ib]
                                for cb in range(4, 8):
                                    ct = work.tile([128, 128], bf16, tag="ct",
                                                   name="ct")
                                    nc.sync.dma_start(
                                        ct[:],
                                        ctx_sc[cb, :,
                                               ib * 128:(ib + 1) * 128])
                                    nc.tensor.matmul(
                                        a0[:], ct[:], wpc[cb][:, 0:512],
                                        start=False, stop=(cb == 7))
                                    nc.tensor.matmul(
                                        a1[:], ct[:], wpc[cb][:, 512:1024],
                                        start=False, stop=(cb == 7))
                                for half, acc in ((0, a0), (1, a1)):
                                    ot = work.tile([128, 512], f32, tag="ot",
                                                   name="ot")
                                    nc.vector.tensor_add(
                                        ot[:], acc[:],
                                        bp_b[:, half * 512:(half + 1) * 512])
                                    nc.sync.dma_start(
                                        out_ap[ib * 128:(ib + 1) * 128,
                                               half * 512:(half + 1) * 512],
                                        ot[:])
                            return f

                        S.append(mk_a())
                        S.append(mk_b())
                    return S

                def attention(hp, steps_by_ip):
                    qzs, kh, vs = state[hp]
                    ctx = {}
                    for ip in range(NQC):
                        qsl = slice(ip * 512, (ip + 1) * 512)
                        steps = steps_by_ip[ip]
                        k = 0
                        for jp in range(16):
                            sc = {}
                            ee = {}
                            for h in range(2):
                                sc[h] = scp.tile([128, 2, 512], f32,
                                                 tag="sc", name="sc")
                                for jj in range(2):
                                    j = jp * 2 + jj
                                    nc.tensor.matmul(
                                        sc[h][:, jj, :],
                                        kh[:, j * 128:(j + 1) * 128],
                                        qzs[h][:, qsl],
                                        start=True, stop=True)
                            for h in range(2):
                                ee[h] = exps.tile([128, 2, 512], bf16,
                                                  tag="e", name="e")
                                nc.scalar.activation(ee[h][:], sc[h][:],
                                                     AF.Exp, scale=0.125)
                            for h in range(2):
                                if jp == 0:
                                    ctx[h] = ctxp.tile([65, 512], f32,
                                                       tag="ctx",
                                                       name=f"ctx{h}")
                                for jj in range(2):
                                    j = jp * 2 + jj
                                    nc.tensor.matmul(
                                        ctx[h][:], vs[:, j, h, :],
                                        ee[h][:, jj, :], start=(j == 0),
                                        stop=(j == 31))
                            if jp == 15:
                                for h in range(2):
                                    ctx_epilogue(hp, ip, h, ctx[h])
                            while k < len(steps) * (jp + 1) // 16:
                                steps[k]()
                                k += 1

                def split4(S):
                    # spread one step list across the 4 ip sub-loops
                    return [S[len(S) * i // 4: len(S) * (i + 1) // 4]
                            for i in range(4)]

                for f in proj_steps(0):
                    f()
                for hp in range(NPAIR):
                    if hp + 1 < NPAIR:
                        steps_by_ip = split4(proj_steps(hp + 1))
                    else:
                        steps_by_ip = [wpc_steps(), outproj_steps(0),
                                       outproj_steps(1), outproj_steps(2)]
                    attention(hp, steps_by_ip)
                for f in outproj_steps(3, alt_pool=True):
                    f()

    nc.compile()
    return nc


def _host_prep(x, W_qkv, q_scale, k_scale, W_proj, b_proj, cos, sin):
    nbf = ml_dtypes.bfloat16
    cosn = np.asarray(cos, np.float32)
    sinn = np.asarray(sin, np.float32)
    qs = np.asarray(q_scale, np.float32)
    ks = np.asarray(k_scale, np.float32)

    def tables(g):
        sign = np.concatenate([-np.ones(D // 2), np.ones(D // 2)]).astype(np.float32)
        gpart = np.concatenate([g[D // 2:], g[:D // 2]])
        cg = cosn * g[None, :]
        sg = sinn * (sign * gpart)[None, :]
        return cg.T.copy(), sg.T.copy()  # [D, L]

    cgq_f, sgq_f = tables(qs)
    cgk_f, sgk_f = tables(ks)

    def b128(t):  # [64, n] -> [128, n] (two heads stacked)
        return np.ascontiguousarray(np.concatenate([t, t], axis=0)).astype(nbf)

    wT = np.asarray(W_qkv, np.float32).T.astype(nbf)
    wpT = np.asarray(W_proj, np.float32).T.astype(nbf)
    bp = np.asarray(b_proj, np.float32).reshape(1, C)
    onesAB = np.zeros((128, 2), nbf)
    onesAB[0:64, 0] = 1.0
    onesAB[64:128, 1] = 1.0

    xn = np.asarray(x, np.float32)
    in_maps = []
    for core in range(NCORES):
        b, qh = core // 2, core % 2
        own = slice(qh * LQ, (qh + 1) * LQ)
        perm = np.r_[np.arange(qh * LQ, (qh + 1) * LQ),
                     np.arange((1 - qh) * LQ, (2 - qh) * LQ)]
        xTc = xn[b].T[:, perm].astype(nbf)
        in_maps.append({
            "xT": np.ascontiguousarray(xTc),
            "wT": wT, "wpT": wpT,
            "cgq": b128(cgq_f[:, own]),
            "sgq": b128(sgq_f[:, own]),
            "cgk": b128(cgk_f[:, perm]),
            "sgk": b128(sgk_f[:, perm]),
            "onesAB": onesAB, "bproj": bp,
        })
    return in_maps


def kernel(x, W_qkv, q_scale, k_scale, W_proj, b_proj, cos, sin, _trace=False):
    global _compiled
    from concourse.bass_utils import run_bass_kernel_spmd
    if _compiled is None:
        _compiled = _build()
    in_maps = _host_prep(x, W_qkv, q_scale, k_scale, W_proj, b_proj, cos, sin)
    res = run_bass_kernel_spmd(_compiled, in_maps, core_ids=list(range(NCORES)),
                               trace=_trace)
    out = np.empty((B, L, C), np.float32)
    for core in range(NCORES):
        b, qh = core // 2, core % 2
        out[b, qh * LQ:(qh + 1) * LQ, :] = res.results[core]["out"]
    kernel._last = res
    return out
